# revision 4
# baseline (speedup 1.0000x reference)
"""Trainium2 Bass kernel for nn_CGEBlock (Clifford Group Equivariant block, Cl(3,0)).

v2: all-fp16 pipeline, host-pre-transposed x, k-major geometric product with
XOR-butterfly reduction (5 large DVE ops per group instead of ~55 small ones),
merged Act ops, fp16 output (host converts to f32).

Blades in *mask order* (blade index == bitmask). Feature layout blade-major:
col = jm*32 + n. Data parallel over 8 cores.
"""

import sys

for p in ("/opt/trn_rl_repo",):
    if p not in sys.path:
        sys.path.insert(0, p)

import numpy as np

import concourse.bass as bass
import concourse.bacc as bacc
import concourse.mybir as mybir
import concourse.tile as tile
from concourse.bass_utils import run_bass_kernel_spmd
from concourse.masks import make_identity

EPS = 1e-6
N_CORES = 8
B_TOTAL = 131072
B_PC = B_TOTAL // N_CORES  # 16384
FIN = 16
FOUT = 32

MASKS = [0, 1, 2, 4, 3, 5, 6, 7]  # reference blade idx -> mask (self-inverse)
GRADE_IDX = [0, 1, 1, 1, 2, 2, 2, 3]
PC = [bin(m).count("1") for m in range(8)]

F32 = mybir.dt.float32
F16 = mybir.dt.float16
AX = mybir.AxisListType
ALU = mybir.AluOpType
AF = mybir.ActivationFunctionType


def _cayley_sign(a, b):
    s, aa = 0, a >> 1
    while aa:
        s += bin(aa & b).count("1")
        aa >>= 1
    return -1.0 if (s & 1) else 1.0


def build_consts(w1, b1, a_relu, b_relu, wl, bl, wr, a_norm, gp_w, a_ln):
    """Host-side constant matrices, fp16, mask-order blade-major columns."""
    c = {}
    isq2 = 1.0 / np.sqrt(2.0)

    # W1big [128=(m,i_ref), 256=(jm,n)]
    W1 = np.zeros((128, 256), np.float32)
    for m in range(FIN):
        for ii in range(8):
            jm = MASKS[ii]
            for n in range(FOUT):
                W1[m * 8 + ii, jm * 32 + n] = w1[n, m, GRADE_IDX[ii]]
    c["W1big"] = W1

    # WWA/WWB [128 rows=(jm,n) half, 512 cols = xr(256) | hl(256)]
    WWA = np.zeros((128, 512), np.float32)
    WWB = np.zeros((128, 512), np.float32)
    for jm in range(8):
        g = PC[jm]
        half, base = (WWA, jm * 32) if jm < 4 else (WWB, (jm - 4) * 32)
        for n in range(FOUT):
            for n2 in range(FOUT):
                half[base + n, jm * 32 + n2] = wr[n2, n, g]
                half[base + n, 256 + jm * 32 + n2] = wl[n2, n, g] * a_ln[n2] * isq2
    c["WWA"] = WWA
    c["WWB"] = WWB

    rep = lambda v: np.repeat(v[None, :].astype(np.float32), 128, 0)
    c["b1row"] = rep(b1)
    c["blrow"] = rep(bl * a_ln * isq2)
    c["invalnr"] = rep(1.0 / a_ln)

    # gate / norm rows, g-major layout: col = g*32 + n
    c["arelur"] = rep(a_relu.T.reshape(-1))
    c["brelur"] = rep(b_relu.T.reshape(-1))
    sig = 1.0 / (1.0 + np.exp(-a_norm))
    c["signr"] = rep(sig.T.reshape(-1))
    c["bias2r"] = rep((1.0 - sig + EPS).T.reshape(-1))

    # wrowsK [128, 2048], k-major: col = i*256 + k*32 + n
    # value = s(i, i^k) * gp_w[n, g(i), g(i^k), g(k)] * a_ln[n] / sqrt(2)
    W = np.zeros((8, 8, FOUT), np.float32)
    for i in range(8):
        for k in range(8):
            j = i ^ k
            s = _cayley_sign(i, j)
            W[i, k, :] = s * gp_w[:, PC[i], PC[j], PC[k]] * a_ln * isq2
    c["wrowsK"] = np.repeat(W.reshape(1, -1), 128, 0)
    return c


CONST_SHAPES = {
    "W1big": (128, 256),
    "WWA": (128, 512),
    "WWB": (128, 512),
    "b1row": (128, 32),
    "blrow": (128, 32),
    "invalnr": (128, 32),
    "arelur": (128, 128),
    "brelur": (128, 128),
    "signr": (128, 128),
    "bias2r": (128, 128),
    "wrowsK": (128, 2048),
}


def _ap(t, off, levels):
    """Custom free-dim AP on tile t: keep partition level, replace free levels."""
    a = t[:]
    return bass.AP(tensor=a.tensor, offset=a.offset + off, ap=[list(a.ap[0])] + levels)


# contiguous mask-order runs sharing one grade: (grade, j0, run_len)
GRUNS = [(0, 0, 1), (1, 1, 2), (2, 3, 1), (1, 4, 1), (2, 5, 2), (3, 7, 1)]


def build_program(b_pc=B_PC):
    nc = bacc.Bacc()
    x_d = nc.dram_tensor("x", [b_pc, 128], F16, kind="ExternalInput")
    out_d = nc.dram_tensor("out", [b_pc, 256], F16, kind="ExternalOutput")
    cd = {k: nc.dram_tensor(k, list(s), F16, kind="ExternalInput")
          for k, s in CONST_SHAPES.items()}

    n_grp = b_pc // 512
    # x shipped as [n_grp, 4s, 128f, 128p] flattened to [(g s f), p]
    xv = x_d[:].rearrange("(g s f) p -> f g s p", s=4, f=128)
    ov = out_d[:].rearrange("(g s p) f -> p g s f", s=4, p=128)

    with tile.TileContext(nc) as tc:
        with (
            tc.tile_pool(name="consts", bufs=1) as consts,
            tc.tile_pool(name="io", bufs=3) as io,
            tc.tile_pool(name="work", bufs=2) as work,
            tc.tile_pool(name="gp", bufs=2) as gpool,
            tc.tile_pool(name="ps", bufs=1, space="PSUM") as ps,
        ):
            C = {}
            for k, s in CONST_SHAPES.items():
                C[k] = consts.tile(list(s), F16, name=k, tag=k)
                nc.sync.dma_start(out=C[k], in_=cd[k][:])
            ident = consts.tile([128, 128], F32)
            make_identity(nc, ident)
            ident16 = consts.tile([128, 128], F16)
            nc.vector.tensor_copy(ident16[:], ident[:])

            for g in range(n_grp):
                xq = io.tile([128, 4, 128], F16)
                nc.sync.dma_start(out=xq, in_=xv[:, g, :, :])
                outq = io.tile([128, 4, 256], F16)

                # ---- h = mvlinear1(x) : xq already transposed on host ----
                h_ps = ps.tile([128, 4, 256], F32, bufs=1, tag="h_ps")
                for s in range(4):
                    nc.tensor.matmul(h_ps[:, s, :], lhsT=xq[:, s, :],
                                     rhs=C["W1big"][:], start=True, stop=True)
                h16 = work.tile([128, 4, 256], F16, bufs=2, tag="h16")
                nc.scalar.activation(h16[:], h_ps[:], AF.Copy)
                nc.vector.tensor_tensor(
                    _ap(h16, 0, [[256, 4], [1, 32]]),
                    _ap(h16, 0, [[256, 4], [1, 32]]),
                    _ap(C["b1row"], 0, [[0, 4], [1, 32]]), ALU.add)
                h2 = work.tile([128, 4, 256], F16, bufs=2, tag="h2")
                nc.vector.tensor_tensor(h2[:], h16[:], h16[:], ALU.mult)

                # ---- invariants [4s,4g,32c] g-major (Pool engine) ----
                inv = work.tile([128, 4, 128], F16, bufs=2, tag="inv")
                iap = lambda t, j: _ap(t, j * 32, [[256 if t is not inv else 128, 4],
                                                   [1, 32]])
                nc.gpsimd.tensor_copy(iap(inv, 0), iap(h16, 0))
                nc.gpsimd.tensor_tensor(iap(inv, 1), iap(h2, 1), iap(h2, 2), ALU.add)
                nc.gpsimd.tensor_tensor(iap(inv, 1), iap(inv, 1), iap(h2, 4), ALU.add)
                nc.gpsimd.tensor_tensor(iap(inv, 2), iap(h2, 3), iap(h2, 5), ALU.add)
                nc.gpsimd.tensor_tensor(iap(inv, 2), iap(inv, 2), iap(h2, 6), ALU.add)
                nc.gpsimd.tensor_copy(iap(inv, 3), iap(h2, 7))

                # ---- gates: relu(a*inv + b), g-major [4s,128] ----
                gate = work.tile([128, 4, 128], F16, bufs=2, tag="gate")
                arl = _ap(C["arelur"], 0, [[0, 4], [1, 128]])
                brl = _ap(C["brelur"], 0, [[0, 4], [1, 128]])
                nc.vector.tensor_tensor(gate[:], inv[:], arl, ALU.mult)
                nc.vector.tensor_tensor(gate[:], gate[:], brl, ALU.add)
                nc.vector.tensor_scalar_max(gate[:], gate[:], 0.0)

                # ---- hg = gate[grade-expanded] * h16 (6 grade-run ops) ----
                hg = work.tile([128, 4, 256], F16, bufs=2, tag="hg")
                for grade, j0, ln in GRUNS:
                    nc.vector.tensor_tensor(
                        _ap(hg, j0 * 32, [[256, 4], [32, ln], [1, 32]]),
                        _ap(h16, j0 * 32, [[256, 4], [32, ln], [1, 32]]),
                        _ap(gate, grade * 32, [[128, 4], [0, ln], [1, 32]]),
                        ALU.mult)

                # ---- transposes of hg halves for Wr|Wl matmul ----
                hgT_ps = ps.tile([128, 4, 2, 128], F16, bufs=2, tag="hgT")
                for s in range(4):
                    nc.tensor.transpose(hgT_ps[:, s, 0, :], hg[:, s, 0:128], ident16[:])
                    nc.tensor.transpose(hgT_ps[:, s, 1, :], hg[:, s, 128:256], ident16[:])
                hgTs = work.tile([128, 4, 2, 128], F16, bufs=2, tag="hgTs")
                nc.scalar.activation(hgTs[:], hgT_ps[:], AF.Copy)

                xrhl_ps = ps.tile([128, 4, 512], F32, bufs=1, tag="xrhl")
                for s in range(4):
                    nc.tensor.matmul(xrhl_ps[:, s, :], lhsT=hgTs[:, s, 0, :],
                                     rhs=C["WWA"][:], start=True, stop=False)
                    nc.tensor.matmul(xrhl_ps[:, s, :], lhsT=hgTs[:, s, 1, :],
                                     rhs=C["WWB"][:], start=False, stop=True)
                xrhl = work.tile([128, 4, 512], F16, bufs=2, tag="xrhl16")
                nc.scalar.activation(xrhl[:], xrhl_ps[:], AF.Copy)

                # ---- steerable norms ----
                xr2 = work.tile([128, 4, 256], F16, bufs=2, tag="xr2")
                nc.vector.tensor_tensor(
                    _ap(xr2, 0, [[256, 4], [1, 256]]),
                    _ap(xrhl, 0, [[512, 4], [1, 256]]),
                    _ap(xrhl, 0, [[512, 4], [1, 256]]), ALU.mult)
                qs = work.tile([128, 4, 128], F16, bufs=2, tag="qs")
                qap = lambda t, j: _ap(t, j * 32, [[256 if t is xr2 else 128, 4],
                                                   [1, 32]])
                nc.gpsimd.tensor_copy(qap(qs, 0), qap(xr2, 0))
                nc.gpsimd.tensor_tensor(qap(qs, 1), qap(xr2, 1), qap(xr2, 2), ALU.add)
                nc.gpsimd.tensor_tensor(qap(qs, 1), qap(qs, 1), qap(xr2, 4), ALU.add)
                nc.gpsimd.tensor_tensor(qap(qs, 2), qap(xr2, 3), qap(xr2, 5), ALU.add)
                nc.gpsimd.tensor_tensor(qap(qs, 2), qap(qs, 2), qap(xr2, 6), ALU.add)
                nc.gpsimd.tensor_copy(qap(qs, 3), qap(xr2, 7))
                nt = work.tile([128, 4, 128], F16, bufs=2, tag="nt")
                nc.scalar.activation(nt[:], qs[:], AF.Sqrt)
                den = work.tile([128, 4, 128], F16, bufs=2, tag="den")
                sgr = _ap(C["signr"], 0, [[0, 4], [1, 128]])
                b2r = _ap(C["bias2r"], 0, [[0, 4], [1, 128]])
                nc.vector.tensor_tensor(den[:], nt[:], sgr, ALU.mult)
                nc.vector.tensor_tensor(den[:], den[:], b2r, ALU.add)
                lden = work.tile([128, 4, 128], F32, bufs=2, tag="lden")
                nc.scalar.activation(lden[:], den[:], AF.Ln)
                rden = work.tile([128, 4, 128], F16, bufs=2, tag="rden")
                nc.scalar.activation(rden[:], lden[:], AF.Exp, scale=-1.0)

                # ---- xrn = xr * rden[grade-expanded] (6 grade-run ops) ----
                xrn = work.tile([128, 4, 256], F16, bufs=2, tag="xrn")
                for grade, j0, ln in GRUNS:
                    nc.vector.tensor_tensor(
                        _ap(xrn, j0 * 32, [[256, 4], [32, ln], [1, 32]]),
                        _ap(xrhl, j0 * 32, [[512, 4], [32, ln], [1, 32]]),
                        _ap(rden, grade * 32, [[128, 4], [0, ln], [1, 32]]),
                        ALU.mult)

                # ---- geometric product: k-major V, XOR-butterfly reduce ----
                V = gpool.tile([128, 4, 2048], F16, bufs=2, tag="V")
                nc.vector.tensor_tensor(
                    _ap(V, 0, [[1, 8192]]),
                    _ap(C["wrowsK"], 0, [[0, 4], [1, 2048]]),
                    _ap(xrn, 0, [[256, 4], [0, 8], [1, 256]]), ALU.mult)
                nc.vector.tensor_tensor(
                    _ap(V, 0, [[1, 8192]]),
                    _ap(V, 0, [[1, 8192]]),
                    _ap(hg, 0, [[32, 32], [0, 8], [1, 32]]), ALU.mult)
                geo = work.tile([128, 4, 256], F16, bufs=2, tag="geo")
                # butterfly folds batched over s, split by the XOR-flipped
                # jm bit so every AP stays within 3 free levels
                # F1: V[s, i, m] += V[s, i+4, m^4]  (i<4; split on jm bit2)
                for t in (0, 1):
                    nc.vector.tensor_tensor(
                        _ap(V, t * 128, [[2048, 4], [256, 4], [1, 128]]),
                        _ap(V, t * 128, [[2048, 4], [256, 4], [1, 128]]),
                        _ap(V, 4 * 256 + (1 - t) * 128,
                            [[2048, 4], [256, 4], [1, 128]]), ALU.add)
                # F2: V[s, i, m] += V[s, i+2, m^2]  (i<2; split on jm bit1)
                for t in (0, 1):
                    nc.vector.tensor_tensor(
                        _ap(V, t * 64, [[2048, 4], [128, 4], [1, 64]]),
                        _ap(V, t * 64, [[2048, 4], [128, 4], [1, 64]]),
                        _ap(V, 2 * 256 + (1 - t) * 64,
                            [[2048, 4], [128, 4], [1, 64]]), ALU.add)
                # F3: geo[s, m] = V[s, 0, m] + V[s, 1, m^1]  (split on jm bit0)
                for t in (0, 1):
                    nc.vector.tensor_tensor(
                        _ap(geo, t * 32, [[256, 4], [64, 4], [1, 32]]),
                        _ap(V, t * 32, [[2048, 4], [64, 4], [1, 32]]),
                        _ap(V, 256 + (1 - t) * 32,
                            [[2048, 4], [64, 4], [1, 32]]), ALU.add)

                # ---- hf = hl + geo (+ bl) ----
                hf = work.tile([128, 4, 256], F16, bufs=2, tag="hf")
                nc.vector.tensor_tensor(
                    _ap(hf, 0, [[256, 4], [1, 256]]),
                    _ap(xrhl, 256, [[512, 4], [1, 256]]),
                    _ap(geo, 0, [[256, 4], [1, 256]]), ALU.add)
                nc.vector.tensor_tensor(
                    _ap(hf, 0, [[256, 4], [1, 32]]),
                    _ap(hf, 0, [[256, 4], [1, 32]]),
                    _ap(C["blrow"], 0, [[0, 4], [1, 32]]), ALU.add)

                # ---- MVLayerNorm ----
                hf2 = work.tile([128, 4, 256], F32, bufs=2, tag="hf2")
                nc.scalar.activation(hf2[:], hf[:], AF.Square)
                u1 = work.tile([128, 4, 128], F32, bufs=2, tag="u1")
                nc.gpsimd.tensor_add(u1[:], hf2[:, :, 0:128], hf2[:, :, 128:256])
                u2 = work.tile([128, 4, 64], F32, bufs=2, tag="u2")
                nc.gpsimd.tensor_add(u2[:], u1[:, :, 0:64], u1[:, :, 64:128])
                s32 = work.tile([128, 4, 32], F32, bufs=2, tag="s32")
                nc.gpsimd.tensor_add(s32[:], u2[:, :, 0:32], u2[:, :, 32:64])
                cn = work.tile([128, 4, 32], F16, bufs=2, tag="cn")
                nc.scalar.activation(cn[:], s32[:], AF.Sqrt)
                ivr = _ap(C["invalnr"], 0, [[0, 4], [1, 32]])
                nc.vector.tensor_tensor(cn[:], cn[:], ivr, ALU.mult)
                snrm = work.tile([128, 4], F32, bufs=2, tag="snrm")
                nc.vector.tensor_reduce(snrm[:].unsqueeze(2), cn[:],
                                        axis=AX.X, op=ALU.add)
                den2 = work.tile([128, 4], F32, bufs=2, tag="den2")
                nc.vector.tensor_scalar(den2[:], snrm[:], 1.0 / 32.0, EPS,
                                        op0=ALU.mult, op1=ALU.add)
                rr = work.tile([128, 4], F32, bufs=2, tag="rr")
                rsc = work.tile([128, 4], F32, bufs=2, tag="rsc")
                nc.vector.reciprocal_approx_accurate(rr[:], den2[:], rsc[:])
                rr16 = work.tile([128, 4], F16, bufs=2, tag="rr16")
                nc.vector.tensor_copy(rr16[:], rr[:])
                nc.vector.tensor_tensor(
                    _ap(outq, 0, [[256, 4], [1, 256]]),
                    _ap(hf, 0, [[256, 4], [1, 256]]),
                    _ap(rr16, 0, [[1, 4], [0, 256]]), ALU.mult)

                nc.sync.dma_start(out=ov[:, g, :, :], in_=outq)
    nc.finalize()
    return nc


_PROG = {}
LAST_RESULT = None


def _get_program(b_pc):
    if b_pc not in _PROG:
        _PROG[b_pc] = build_program(b_pc)
    return _PROG[b_pc]


def kernel(**inputs):
    x = np.asarray(inputs["x"], np.float32)
    consts = build_consts(
        np.asarray(inputs["w1"], np.float32), np.asarray(inputs["b1"], np.float32),
        np.asarray(inputs["a_relu"], np.float32), np.asarray(inputs["b_relu"], np.float32),
        np.asarray(inputs["wl"], np.float32), np.asarray(inputs["bl"], np.float32),
        np.asarray(inputs["wr"], np.float32), np.asarray(inputs["a_norm"], np.float32),
        np.asarray(inputs["gp_w"], np.float32), np.asarray(inputs["a_ln"], np.float32),
    )
    consts = {k: v.astype(np.float16) for k, v in consts.items()}
    b_total = x.shape[0]
    b_pc = b_total // N_CORES
    n_grp = b_pc // 512
    nc = _get_program(b_pc)
    # host: fp16, transposed per subtile (W1big rows handle blade mapping)
    xm = x.reshape(b_total, 128).astype(np.float16)
    in_maps = []
    for c in range(N_CORES):
        xc = xm[c * b_pc:(c + 1) * b_pc]
        xT = np.ascontiguousarray(
            xc.reshape(n_grp, 4, 128, 128).transpose(0, 1, 3, 2)
        ).reshape(n_grp * 512, 128)
        m = {"x": xT}
        m.update(consts)
        in_maps.append(m)
    import os
    trace = os.environ.get("KERNEL_TRACE", "0") == "1"
    res = run_bass_kernel_spmd(nc, in_maps, core_ids=list(range(N_CORES)),
                               trace=trace)
    global LAST_RESULT
    LAST_RESULT = res
    outs = [
        res.results[c]["out"].astype(np.float32).reshape(b_pc, 8, FOUT)[:, MASKS, :]
        .transpose(0, 2, 1)
        for c in range(N_CORES)
    ]
    return np.ascontiguousarray(np.concatenate(outs, axis=0).astype(np.float32))


if __name__ == "__main__":
    print("building program...")
    build_program(512)
    print("ok")
